# revision 108
# baseline (speedup 1.0000x reference)
"""Trainium2 Bass kernel for nn_MultiHeadAttention_37623913513495.

Multi-head attention with rotary embeddings and a relative-position bias
(einsum('bhid,ijd->bhij', q, rel_pos[j-i+T-1])), sharded over 8 NeuronCores
as 4 batches x 2 head-groups (8 heads each). Host sums the two partial
outputs per batch and adds the bias.

Device-side structure (per core):
  - all inputs shipped bf16 (x pre-transposed to xT [C, T]); q/k
    projections produce qT/kT [d, T], v [T, d]; rotary in bf16 on DVE
    with the mul/add half offloaded to GPSIMD; 1/sqrt(hs) folded into wq
    on the host (rel bias is unscaled in the reference, so E is shipped
    pre-multiplied by 8); cos/sin packed into one bf16 table
  - the relative-position "skew" rel[i,j] = A[i, j-i+c] is realized by
    writing raw A windows to DRAM (bf16) and re-reading them with a
    strided diagonal access pattern [[W-1, 128], [1, T]]
  - the skewed bias is accumulated into the score PSUM via an identity
    matmul (emitted after both score matmuls so scores never wait on the
    skew round trip); a single Exp on ACT emits P and the row sums via
    accum_out; P is transposed and normalized in one matmul against
    diag(1/rowsum); out-projection in bf16, y written back as bf16
  - DMA: per-chunk xT/wq loads overlap the first projection matmuls;
    wk/wv/wo ride single batched 3D-AP DMAs; a_dr writes + skew reads on
    the SP ring, y on the ACT ring, tables on SWDGE
"""

import numpy as np

HS = 64           # head size
NH = 16           # total heads
SEQ = 1024        # sequence length
EMB = 1024        # embedding dim
BATCH = 4
N_CORES = 8
HC = NH // 2      # heads per core

_cache = {}


def _build_nc(T, C, D, reps=1):
    import concourse.bass as bass
    import concourse.bacc as bacc
    import concourse.mybir as mybir
    import concourse.tile as tile
    from concourse.masks import make_identity

    dt = mybir.dt
    f32, f32r, bf16 = dt.float32, dt.float32r, dt.bfloat16
    fp8 = dt.float8e4
    AF = mybir.ActivationFunctionType

    P = 128
    NB = T // P              # row blocks
    KC = C // P              # contraction chunks over C
    DT = D // P              # qT/kT partition tiles
    HPT = P // HS            # heads per qT tile (2)
    HCL = D // HS            # heads on this core
    JH = min(512, T)         # j-half width
    NJH = T // JH            # j-halves per row
    W = JH + P               # A-window width
    WH = W // 2              # A psum tile width
    NCH = T // P             # j chunks for PT/AV
    PTG = 512 // P           # PT chunks per psum tile

    nc = bacc.Bacc(None, target_bir_lowering=False, debug=False)

    xT_d = nc.dram_tensor("xT", [C, T], bf16, kind="ExternalInput")
    wq_d = nc.dram_tensor("wq", [C, D], bf16, kind="ExternalInput")
    wk_d = nc.dram_tensor("wk", [C, D], bf16, kind="ExternalInput")
    wv_d = nc.dram_tensor("wv", [C, D], bf16, kind="ExternalInput")
    wo_d = nc.dram_tensor("wo", [D, C], bf16, kind="ExternalInput")
    cs_d = nc.dram_tensor("csT", [P, 2 * T], bf16, kind="ExternalInput")
    et_d = nc.dram_tensor("et8", [P, 2 * T], bf16, kind="ExternalInput")
    y_d = nc.dram_tensor("y", [T, C], bf16, kind="ExternalOutput")

    with tile.TileContext(nc) as tc:
        with (
            tc.tile_pool(name="const", bufs=1) as const,
            tc.tile_pool(name="persist", bufs=1) as persist,
            tc.tile_pool(name="asb", bufs=5) as asb_pool,
            tc.tile_pool(name="relsb", bufs=NB + 16) as rel_pool,
            tc.tile_pool(name="psb", bufs=4) as p_pool,
            tc.tile_pool(name="ptsb", bufs=5) as pt_pool,
            tc.tile_pool(name="small", bufs=8) as small,
            tc.tile_pool(name="outsb", bufs=4) as out_pool,
            tc.tile_pool(name="ps_s", bufs=2, space="PSUM") as ps_s,
            tc.tile_pool(name="ps_pt", bufs=3, space="PSUM") as ps_pt,
            tc.tile_pool(name="ps_av", bufs=1, space="PSUM") as ps_av,
            tc.tile_pool(name="adram", bufs=2 * (NB + 2) * NJH,
                         space="DRAM") as adram,
        ):
            for _rep in range(reps):
                # ---------------- input tiles ----------------
                # allocation order is reverse free order (LIFO pools): wo dies
                # last, then xT/rot_tmp/wv/wk, wq dies first.  Each tensor is
                # fetched as ONE batched DMA (3D AP, [P, kb, cols] view of the
                # row-major DRAM array) to amortize the ~630ns per-DMA HWDGE
                # cost; xT is split in two across the rings.
                def fold_src(dram_t, kn, ncol, k0=0):
                    full = dram_t[:, :]
                    return bass.AP(
                        tensor=full.tensor, offset=full.offset + k0 * P * ncol,
                        ap=[[ncol, P], [P * ncol, kn], [1, ncol]])

                wo_all, _wo_fr = tc.tile([P, DT, C], bf16, name="wo")
                wo_sb = [wo_all[:, hc, :] for hc in range(DT)]
                wo_frees = [_wo_fr]

                # xT per-chunk so the first projection matmuls can start as
                # soon as chunk 0 lands (fine-grained DMA<->PE overlap)
                xT_sb, xT_free = [], []
                for cb in range(KC):
                    t, fr = tc.tile([P, T], bf16, name=f"xT_{cb}")
                    nc.sync.dma_start(out=t, in_=xT_d[cb * P:(cb + 1) * P, :])
                    xT_sb.append(t)
                    xT_free.append(fr)

                # ---------------- constants (gpsimd SWDGE, off the HWDGE rings) --
                ident_b = const.tile([P, P], bf16)
                make_identity(nc, ident_b)
                cs_sb = const.tile([P, 2 * T], bf16)
                nc.gpsimd.dma_start(out=cs_sb, in_=cs_d[:, :])
                cos_sb = cs_sb[:, :T]
                sin_sb = cs_sb[:, T:]
                et_bf = const.tile([P, 2 * T], bf16)

                # ---------------- projections ----------------
                def alloc_w(name):
                    t, fr = tc.tile([P, KC, D], bf16, name=name)
                    return t, [t[:, kb, :] for kb in range(KC)], [fr]

                # d-pairs (d, d+32) are laid out 16 apart within a 32-partition
                # quadrant (host permutes weights/E/tables to match), so the
                # rotary "rotate_half" partner sits at p^16 — reachable by DVE
                # stream_shuffle.
                shuf_mask = [(i + 16) % 32 for i in range(32)]

                qT_sb = [persist.tile([P, T], bf16, name=f"qT_{mb}", tag=f"qT{mb}")
                         for mb in range(DT)]
                kT_sb = [persist.tile([P, T], bf16, name=f"kT_{mb}", tag=f"kT{mb}")
                         for mb in range(DT)]
                rot_tmp0, rot_tmp0_free = tc.tile([P, T], bf16, name="rot_tmp0")
                rot_tmp1, rot_tmp1_free = tc.tile([P, T], bf16, name="rot_tmp1")
                rot_tmps = [rot_tmp0, rot_tmp1]

                WF = T + P               # full A-window width per row-block
                a_chunks = [512] * (WF // 512)
                if WF % 512:
                    a_chunks.append(WF % 512)

                def emit_A_block(h, ib, write_eng=None):
                    """A window for (head h, row-block ib) -> DRAM -> skewed read."""
                    par = (h % HPT) * HS
                    i0 = ib * P
                    w0 = (T - P) - i0
                    lhs_bf = qT_sb[h // HPT][par:par + HS, i0:i0 + P]
                    a_sb = asb_pool.tile([P, WF], bf16, name="a_sb", tag="a_sb")
                    off = 0
                    for ci, cw in enumerate(a_chunks):
                        a_ps = ps_pt.tile([P, cw], f32, name="a_ps", tag="ptp")
                        nc.tensor.matmul(
                            a_ps, lhs_bf,
                            et_bf[par:par + HS, w0 + off:w0 + off + cw],
                            start=True, stop=True)
                        if (h + ib + ci) % 2 == 0:
                            nc.scalar.copy(a_sb[:, off:off + cw], a_ps)
                        else:
                            nc.vector.tensor_copy(a_sb[:, off:off + cw], a_ps)
                        off += cw
                    a_dr = adram.tile([P, WF], bf16, name="a_dr", tag="a_dr")
                    (write_eng or nc.sync).dma_start(out=a_dr, in_=a_sb)
                    # defer the skew read one A-block: its SP-SEQ park then
                    # overlaps the NEXT write's transfer instead of stalling it
                    read_q.append((h, ib, a_dr))
                    if len(read_q) > 1:
                        emit_A_read()

                def emit_A_read():
                    h, ib, a_dr = read_q.pop(0)
                    rel = rel_pool.tile([P, T], bf16, name="rel", tag="rel")
                    skew = bass.AP(
                        tensor=a_dr.tensor,
                        offset=a_dr.offset + (P - 1),
                        ap=[[WF - 1, P], [1, T]],
                    )
                    nc.sync.dma_start(out=rel, in_=skew)
                    rel_tiles[(h, ib)] = rel

                rel_tiles = {}
                stash = {}
                read_q = []

                def emit_attn_SE(h, ib):
                    """scores + rel add + exp + 1/rowsum + diag for block ib."""
                    par = (h % HPT) * HS
                    qtile, ktile = qT_sb[h // HPT], kT_sb[h // HPT]
                    i0 = ib * P
                    p_sb = p_pool.tile([P, T], bf16, name="p_sb", tag="p_sb", bufs=9)
                    s_ps = ps_s.tile([P, T], f32, name="s_ps", tag="s")
                    while (h, ib) not in rel_tiles and read_q:
                        emit_A_read()
                    rel = rel_tiles.pop((h, ib))
                    for jh in range(NJH):
                        sl = slice(jh * JH, (jh + 1) * JH)
                        nc.tensor.matmul(
                            s_ps[:, sl],
                            qtile[par:par + HS, i0:i0 + P],
                            ktile[par:par + HS, sl],
                            start=True, stop=False)
                    for jh in range(NJH):
                        sl = slice(jh * JH, (jh + 1) * JH)
                        # accumulate the skewed rel-bias into the score PSUM
                        nc.tensor.matmul(
                            s_ps[:, sl], ident_b, rel[:, sl],
                            start=False, stop=True)
                    sums_c = small.tile([P, 1], f32, name="sums", tag="sums",
                                        bufs=9)
                    # P = exp(S + rel); row-sums fused via ACT accumulator
                    nc.scalar.activation(p_sb, s_ps, AF.Exp, accum_out=sums_c)
                    rec = small.tile([P, 1], f32, name="rec", tag="rec", bufs=9)
                    nc.vector.reciprocal(rec, sums_c)
                    dg = small.tile([P, P], bf16, name="dg", tag="dg", bufs=9)
                    nc.vector.tensor_scalar_mul(dg, ident_b, rec)
                    stash[(h, ib)] = (p_sb, dg)

                def emit_attn_PTAV(h, ib):
                    """scaled transpose of P + AV accumulation for block ib."""
                    par = (h % HPT) * HS
                    i0 = ib * P
                    p_sb, dg = stash.pop((h, ib))
                    pt_sb = pt_pool.tile([P, NCH, P], bf16, name="pt_sb", tag="pt_sb")
                    for g in range((NCH + PTG - 1) // PTG):
                        gn = min(PTG, NCH - g * PTG)
                        ptp = ps_pt.tile([P, gn * P], f32, name="ptp", tag="ptp")
                        for c in range(gn):
                            jc = g * PTG + c
                            nc.tensor.matmul(
                                ptp[:, c * P:(c + 1) * P],
                                p_sb[:, jc * P:(jc + 1) * P], dg,
                                start=True, stop=True)
                        dst = pt_sb[:, g * PTG:g * PTG + gn, :].rearrange(
                            "p a b -> p (a b)")
                        # ~25% of PT psum->sbuf moves on ACT, rest on DVE
                        if g == 0 and ib % 2 == 0:
                            nc.scalar.copy(dst, ptp)
                        else:
                            nc.vector.tensor_copy(dst, ptp)
                    av = ps_av.tile([HS, P], f32, name="av", tag="av")
                    for jc in range(NCH):
                        nc.tensor.matmul(
                            av,
                            v_sb[jc][:, h * HS:(h + 1) * HS],
                            pt_sb[:, jc, :],
                            start=(jc == 0), stop=(jc == NCH - 1))
                    nc.vector.tensor_copy(
                        attnT_sb[h // HPT][par:par + HS, i0:i0 + P], av)

                # all input loads issued up-front, spread across HWDGE rings so
                # later a_dr/skew/y traffic queues behind nothing compute-bound
                wv_all, wv_sb, wv_frees = alloc_w("wv")
                wk_all, wk_sb, wk_frees = alloc_w("wk")
                # wq per-chunk (overlaps with xT chunk arrival), rest batched
                wq_sb, wq_frees = [], []
                for kb in range(KC):
                    t, fr = tc.tile([P, D], bf16, name=f"wq_{kb}")
                    wq_sb.append(t)
                    wq_frees.append(fr)
                nc.scalar.dma_start(out=wq_sb[0], in_=wq_d[0:P, :])
                nc.scalar.dma_start(out=et_bf, in_=et_d[:, :])
                for kb in range(1, KC):
                    nc.scalar.dma_start(out=wq_sb[kb],
                                        in_=wq_d[kb * P:(kb + 1) * P, :])
                nc.sync.dma_start(out=wk_all[:, :, :],
                                  in_=fold_src(wk_d, KC, D))
                nc.scalar.dma_start(out=wv_all[:, :, :],
                                    in_=fold_src(wv_d, KC, D))
                nc.sync.dma_start(out=wo_all[:, :, :],
                                  in_=fold_src(wo_d, DT, C))

                for (wname, w_sb, w_frees, dest) in (
                        ("wq", wq_sb, wq_frees, qT_sb),
                        ("wk", wk_sb, wk_frees, kT_sb)):
                    for mb in range(DT):
                        raw, raw_free = tc.tile([P, T], bf16, name=f"raw{wname}_{mb}")
                        for nh in range(NJH):
                            pp = ps_pt.tile([P, JH], f32, name="projps", tag="ptp")
                            for kb in range(KC):
                                nc.tensor.matmul(
                                    pp,
                                    w_sb[kb][:, mb * P:(mb + 1) * P],
                                    xT_sb[kb][:, nh * JH:(nh + 1) * JH],
                                    start=(kb == 0), stop=(kb == KC - 1),
                                )
                            if wname == "wq":
                                nc.scalar.copy(
                                    raw[:, nh * JH:(nh + 1) * JH], pp)
                            else:
                                nc.vector.tensor_copy(
                                    raw[:, nh * JH:(nh + 1) * JH], pp)
                        # rotary: bf16 throughout; alternate DVE/Pool per tile
                        o = dest[mb]
                        rt = rot_tmps[mb % 2]
                        eng = nc.vector if mb % 2 == 0 else nc.gpsimd
                        nc.vector.stream_shuffle(rt, raw, shuf_mask)
                        eng.tensor_mul(o, raw, cos_sb)
                        eng.tensor_mul(rt, rt, sin_sb)
                        eng.tensor_add(o, o, rt)
                        raw_free()
                    if wname == "wq":
                        for ib in range(NB):
                            emit_A_block(0, ib)
                    for fr in reversed(w_frees):
                        fr()

                # v[t, d] (bf16), with block-0 score/softmax interleaved
                v_sb = [persist.tile([P, D], bf16, name=f"v_{tb}", tag=f"v{tb}")
                        for tb in range(NB)]
                vw = min(JH, D)
                for tb in range(NB):
                    for nh in range(max(D // JH, 1)):
                        pp = ps_pt.tile([P, vw], f32, name="vps", tag="ptp")
                        for kb in range(KC):
                            nc.tensor.matmul(
                                pp,
                                xT_sb[kb][:, tb * P:(tb + 1) * P],
                                wv_sb[kb][:, nh * vw:(nh + 1) * vw],
                                start=(kb == 0), stop=(kb == KC - 1),
                            )
                        nc.vector.tensor_copy(
                            v_sb[tb][:, nh * vw:(nh + 1) * vw], pp)
                    if tb >= 1:
                        emit_attn_SE(0, tb - 1)
                    emit_A_block(1, tb)
                    if tb >= 4:
                        emit_A_block(2, tb - 4)
                for fr in reversed(wv_frees):
                    fr()
                rot_tmp1_free()
                rot_tmp0_free()
                for fr in reversed(xT_free):
                    fr()

                # ---------------- attention ----------------
                attnT_sb = [persist.tile([P, T], bf16, name=f"attnT_{mb}", tag=f"aT{mb}")
                            for mb in range(DT)]

                def emit_out_block(tb):
                    """final projection for row-block tb (all heads done)."""
                    for ch in range(max(C // JH, 1)):
                        cw = min(JH, C)
                        op = ps_pt.tile([P, cw], f32, name="ops", tag="ptp")
                        for hc in range(DT):
                            nc.tensor.matmul(
                                op,
                                attnT_sb[hc][:, tb * P:(tb + 1) * P],
                                wo_sb[hc][:, ch * cw:(ch + 1) * cw],
                                start=(hc == 0), stop=(hc == DT - 1))
                        o_sb = out_pool.tile([P, cw], bf16, name="o_sb", tag="o_sb")
                        nc.vector.tensor_copy(o_sb, op)
                        nc.scalar.dma_start(
                            out=y_d[tb * P:(tb + 1) * P, ch * cw:(ch + 1) * cw],
                            in_=o_sb)



                DELAY = 1
                for h in range(HCL):
                    for ib in range(NB):
                        tgt = h * NB + ib + 2 * NB + 4
                        if tgt < HCL * NB:
                            emit_A_block(tgt // NB, tgt % NB)
                        if h > 0 or ib >= NB - 1:
                            emit_attn_SE(h, ib)
                        if ib >= DELAY:
                            emit_attn_PTAV(h, ib - DELAY)
                            if h == HCL - 1:
                                emit_out_block(ib - DELAY)
                    for ib in range(NB - DELAY, NB):
                        emit_attn_PTAV(h, ib)
                        if h == HCL - 1:
                            emit_out_block(ib)
                for fr in reversed(wo_frees):
                    fr()


    nc.compile()
    return nc


# partition p (within a head's 64) holds head-dim SIGMA[p]; pairs
# (d, d+32) land 16 apart inside a 32-partition quadrant.
SIGMA = np.concatenate([
    np.arange(0, 16), np.arange(32, 48),
    np.arange(16, 32), np.arange(48, 64),
])


def _host_tables(T, hs):
    inv_freq = 1.0 / (10000.0 ** (np.arange(0, hs, 2, dtype=np.float64) / hs))
    t = np.arange(T, dtype=np.float64)
    fr = np.outer(inv_freq, t)                     # [hs/2, T]
    cosT = np.empty((128, T), np.float32)
    sinS = np.empty((128, T), np.float32)
    for blk in range(128 // hs):
        for p in range(hs):
            d = SIGMA[p]
            row = blk * hs + p
            cosT[row] = np.cos(fr[d % 32]).astype(np.float32)
            s = np.sin(fr[d % 32]).astype(np.float32)
            sinS[row] = -s if d < 32 else s
    return cosT, sinS


def make_et8(E, T, scale=8.0):
    et8 = np.zeros((128, 2 * T), np.float32)
    etp = (scale * E.T[SIGMA]).astype(np.float32)   # [64, 2T-1] permuted rows
    et8[:HS, :E.shape[0]] = etp
    et8[HS:2 * HS, :E.shape[0]] = etp
    return et8


def perm_cols(w, D):
    """Permute per-head 64-column blocks of w [C, D] by SIGMA."""
    idx = (np.arange(D) // HS) * HS + SIGMA[np.arange(D) % HS]
    return np.ascontiguousarray(w[:, idx])


def get_nc(T=SEQ, C=EMB, D=HC * HS):
    key = (T, C, D)
    if key not in _cache:
        _cache[key] = _build_nc(T, C, D)
    return _cache[key]


def kernel(x, wq, wk, wv, wo, bo, rel_pos_emb):
    from concourse.bass_utils import run_bass_kernel_spmd

    x = np.asarray(x, dtype=np.float32)
    wq = np.asarray(wq, dtype=np.float32)
    wk = np.asarray(wk, dtype=np.float32)
    wv = np.asarray(wv, dtype=np.float32)
    wo = np.asarray(wo, dtype=np.float32)
    bo = np.asarray(bo, dtype=np.float32)
    E = np.asarray(rel_pos_emb, dtype=np.float32)

    T, C, D = SEQ, EMB, HC * HS
    nc = get_nc(T, C, D)

    cosT, sinS = _host_tables(T, HS)
    et8 = make_et8(E, T)

    import ml_dtypes
    bf16 = ml_dtypes.bfloat16
    in_maps = []
    for core in range(N_CORES):
        b, g = divmod(core, 2)
        sl = slice(g * D, (g + 1) * D)
        in_maps.append({
            "xT": np.ascontiguousarray(x[b].T).astype(bf16),
            "wq": (perm_cols(wq[:, sl], D) * np.float32(0.125)).astype(bf16),
            "wk": perm_cols(wk[:, sl], D).astype(bf16),
            "wv": np.ascontiguousarray(wv[:, sl]).astype(bf16),
            "wo": np.ascontiguousarray(wo[sl, :]).astype(bf16),
            "csT": np.concatenate([cosT, sinS], axis=1).astype(bf16),
            "et8": et8.astype(bf16),
        })

    res = run_bass_kernel_spmd(nc, in_maps, core_ids=list(range(N_CORES)))
    out = np.empty((BATCH, T, C), np.float32)
    for b in range(BATCH):
        out[b] = (np.asarray(res.results[2 * b]["y"], np.float32)
                  + np.asarray(res.results[2 * b + 1]["y"], np.float32) + bo)
    return out



# revision 109
# speedup vs baseline: 1.0297x; 1.0297x over previous
"""Trainium2 Bass kernel for nn_MultiHeadAttention_37623913513495.

Multi-head attention with rotary embeddings and a relative-position bias
(einsum('bhid,ijd->bhij', q, rel_pos[j-i+T-1])), sharded over 8 NeuronCores
as 4 batches x 2 head-groups (8 heads each). Host sums the two partial
outputs per batch and adds the bias.

Device-side structure (per core):
  - all inputs shipped bf16 (x pre-transposed to xT [C, T]); q/k
    projections produce qT/kT [d, T], v [T, d]; rotary in bf16 on DVE
    with the mul/add half offloaded to GPSIMD; 1/sqrt(hs) folded into wq
    on the host (rel bias is unscaled in the reference, so E is shipped
    pre-multiplied by 8); cos/sin packed into one bf16 table
  - the relative-position "skew" rel[i,j] = A[i, j-i+c] is realized by
    writing raw A windows to DRAM (bf16) and re-reading them with a
    strided diagonal access pattern [[W-1, 128], [1, T]]
  - the skewed bias is accumulated into the score PSUM via an identity
    matmul (emitted after both score matmuls so scores never wait on the
    skew round trip); a single Exp on ACT emits P and the row sums via
    accum_out; P is transposed and normalized in one matmul against
    diag(1/rowsum); out-projection in bf16, y written back as bf16
  - DMA: per-chunk xT/wq loads overlap the first projection matmuls;
    wk/wv/wo ride single batched 3D-AP DMAs; a_dr writes + skew reads on
    the SP ring, y on the ACT ring, tables on SWDGE
"""

import numpy as np

HS = 64           # head size
NH = 16           # total heads
SEQ = 1024        # sequence length
EMB = 1024        # embedding dim
BATCH = 4
N_CORES = 8
HC = NH // 2      # heads per core

_cache = {}


def _build_nc(T, C, D, reps=1):
    import concourse.bass as bass
    import concourse.bacc as bacc
    import concourse.mybir as mybir
    import concourse.tile as tile
    from concourse.masks import make_identity

    dt = mybir.dt
    f32, f32r, bf16 = dt.float32, dt.float32r, dt.bfloat16
    fp8 = dt.float8e4
    AF = mybir.ActivationFunctionType

    P = 128
    NB = T // P              # row blocks
    KC = C // P              # contraction chunks over C
    DT = D // P              # qT/kT partition tiles
    HPT = P // HS            # heads per qT tile (2)
    HCL = D // HS            # heads on this core
    JH = min(512, T)         # j-half width
    NJH = T // JH            # j-halves per row
    W = JH + P               # A-window width
    WH = W // 2              # A psum tile width
    NCH = T // P             # j chunks for PT/AV
    PTG = 512 // P           # PT chunks per psum tile

    nc = bacc.Bacc(None, target_bir_lowering=False, debug=False)

    xT_d = nc.dram_tensor("xT", [C, T], bf16, kind="ExternalInput")
    wq_d = nc.dram_tensor("wq", [C, D], bf16, kind="ExternalInput")
    wk_d = nc.dram_tensor("wk", [C, D], bf16, kind="ExternalInput")
    wv_d = nc.dram_tensor("wv", [C, D], bf16, kind="ExternalInput")
    wo_d = nc.dram_tensor("wo", [D, C], bf16, kind="ExternalInput")
    cs_d = nc.dram_tensor("csT", [P, 2 * T], bf16, kind="ExternalInput")
    et_d = nc.dram_tensor("et8", [P, 2 * T], bf16, kind="ExternalInput")
    y_d = nc.dram_tensor("y", [T, C], bf16, kind="ExternalOutput")

    with tile.TileContext(nc) as tc:
        with (
            tc.tile_pool(name="const", bufs=1) as const,
            tc.tile_pool(name="persist", bufs=1) as persist,
            tc.tile_pool(name="asb", bufs=5) as asb_pool,
            tc.tile_pool(name="relsb", bufs=NB + 12) as rel_pool,
            tc.tile_pool(name="psb", bufs=4) as p_pool,
            tc.tile_pool(name="ptsb", bufs=5) as pt_pool,
            tc.tile_pool(name="small", bufs=8) as small,
            tc.tile_pool(name="outsb", bufs=4) as out_pool,
            tc.tile_pool(name="ps_s", bufs=2, space="PSUM") as ps_s,
            tc.tile_pool(name="ps_pt", bufs=3, space="PSUM") as ps_pt,
            tc.tile_pool(name="ps_av", bufs=1, space="PSUM") as ps_av,
            tc.tile_pool(name="adram", bufs=2 * (NB + 2) * NJH,
                         space="DRAM") as adram,
        ):
            for _rep in range(reps):
                # ---------------- input tiles ----------------
                # allocation order is reverse free order (LIFO pools): wo dies
                # last, then xT/rot_tmp/wv/wk, wq dies first.  Each tensor is
                # fetched as ONE batched DMA (3D AP, [P, kb, cols] view of the
                # row-major DRAM array) to amortize the ~630ns per-DMA HWDGE
                # cost; xT is split in two across the rings.
                def fold_src(dram_t, kn, ncol, k0=0):
                    full = dram_t[:, :]
                    return bass.AP(
                        tensor=full.tensor, offset=full.offset + k0 * P * ncol,
                        ap=[[ncol, P], [P * ncol, kn], [1, ncol]])

                wo_all, _wo_fr = tc.tile([P, DT, C], bf16, name="wo")
                wo_sb = [wo_all[:, hc, :] for hc in range(DT)]
                wo_frees = [_wo_fr]

                # xT per-chunk so the first projection matmuls can start as
                # soon as chunk 0 lands (fine-grained DMA<->PE overlap)
                xT_sb, xT_free = [], []
                for cb in range(KC):
                    t, fr = tc.tile([P, T], bf16, name=f"xT_{cb}")
                    nc.sync.dma_start(out=t, in_=xT_d[cb * P:(cb + 1) * P, :])
                    xT_sb.append(t)
                    xT_free.append(fr)

                # ---------------- constants (gpsimd SWDGE, off the HWDGE rings) --
                ident_b = const.tile([P, P], bf16)
                make_identity(nc, ident_b)
                cs_sb = const.tile([P, 2 * T], bf16)
                nc.gpsimd.dma_start(out=cs_sb, in_=cs_d[:, :])
                cos_sb = cs_sb[:, :T]
                sin_sb = cs_sb[:, T:]
                et_bf = const.tile([P, 2 * T], bf16)

                # ---------------- projections ----------------
                def alloc_w(name):
                    t, fr = tc.tile([P, KC, D], bf16, name=name)
                    return t, [t[:, kb, :] for kb in range(KC)], [fr]

                # d-pairs (d, d+32) are laid out 16 apart within a 32-partition
                # quadrant (host permutes weights/E/tables to match), so the
                # rotary "rotate_half" partner sits at p^16 — reachable by DVE
                # stream_shuffle.
                shuf_mask = [(i + 16) % 32 for i in range(32)]

                qT_sb = [persist.tile([P, T], bf16, name=f"qT_{mb}", tag=f"qT{mb}")
                         for mb in range(DT)]
                kT_sb = [persist.tile([P, T], bf16, name=f"kT_{mb}", tag=f"kT{mb}")
                         for mb in range(DT)]
                rot_tmp0, rot_tmp0_free = tc.tile([P, T], bf16, name="rot_tmp0")
                rot_tmp1, rot_tmp1_free = tc.tile([P, T], bf16, name="rot_tmp1")
                rot_tmps = [rot_tmp0, rot_tmp1]

                WF = T + P               # full A-window width per row-block
                a_chunks = [512] * (WF // 512)
                if WF % 512:
                    a_chunks.append(WF % 512)

                def emit_A_block(h, ib, write_eng=None):
                    """A window for (head h, row-block ib) -> DRAM -> skewed read."""
                    par = (h % HPT) * HS
                    i0 = ib * P
                    w0 = (T - P) - i0
                    lhs_bf = qT_sb[h // HPT][par:par + HS, i0:i0 + P]
                    a_sb = asb_pool.tile([P, WF], bf16, name="a_sb", tag="a_sb")
                    off = 0
                    for ci, cw in enumerate(a_chunks):
                        a_ps = ps_pt.tile([P, cw], f32, name="a_ps", tag="ptp")
                        nc.tensor.matmul(
                            a_ps, lhs_bf,
                            et_bf[par:par + HS, w0 + off:w0 + off + cw],
                            start=True, stop=True)
                        if (h + ib + ci) % 2 == 0:
                            nc.scalar.copy(a_sb[:, off:off + cw], a_ps)
                        else:
                            nc.vector.tensor_copy(a_sb[:, off:off + cw], a_ps)
                        off += cw
                    a_dr = adram.tile([P, WF], bf16, name="a_dr", tag="a_dr")
                    (write_eng or nc.sync).dma_start(out=a_dr, in_=a_sb)
                    # defer the skew read one A-block: its SP-SEQ park then
                    # overlaps the NEXT write's transfer instead of stalling it
                    read_q.append((h, ib, a_dr))
                    if len(read_q) > 1:
                        emit_A_read()

                def emit_A_read():
                    h, ib, a_dr = read_q.pop(0)
                    rel = rel_pool.tile([P, T], bf16, name="rel", tag="rel")
                    skew = bass.AP(
                        tensor=a_dr.tensor,
                        offset=a_dr.offset + (P - 1),
                        ap=[[WF - 1, P], [1, T]],
                    )
                    nc.sync.dma_start(out=rel, in_=skew)
                    rel_tiles[(h, ib)] = rel

                rel_tiles = {}
                stash = {}
                read_q = []

                def emit_attn_SE(h, ib):
                    """scores + rel add + exp + 1/rowsum + diag for block ib."""
                    par = (h % HPT) * HS
                    qtile, ktile = qT_sb[h // HPT], kT_sb[h // HPT]
                    i0 = ib * P
                    p_sb = p_pool.tile([P, T], bf16, name="p_sb", tag="p_sb", bufs=9)
                    s_ps = ps_s.tile([P, T], f32, name="s_ps", tag="s")
                    while (h, ib) not in rel_tiles and read_q:
                        emit_A_read()
                    rel = rel_tiles.pop((h, ib))
                    for jh in range(NJH):
                        sl = slice(jh * JH, (jh + 1) * JH)
                        nc.tensor.matmul(
                            s_ps[:, sl],
                            qtile[par:par + HS, i0:i0 + P],
                            ktile[par:par + HS, sl],
                            start=True, stop=False)
                    for jh in range(NJH):
                        sl = slice(jh * JH, (jh + 1) * JH)
                        # accumulate the skewed rel-bias into the score PSUM
                        nc.tensor.matmul(
                            s_ps[:, sl], ident_b, rel[:, sl],
                            start=False, stop=True)
                    sums_c = small.tile([P, 1], f32, name="sums", tag="sums",
                                        bufs=9)
                    # P = exp(S + rel); row-sums fused via ACT accumulator
                    nc.scalar.activation(p_sb, s_ps, AF.Exp, accum_out=sums_c)
                    rec = small.tile([P, 1], f32, name="rec", tag="rec", bufs=9)
                    nc.vector.reciprocal(rec, sums_c)
                    dg = small.tile([P, P], bf16, name="dg", tag="dg", bufs=9)
                    nc.vector.tensor_scalar_mul(dg, ident_b, rec)
                    stash[(h, ib)] = (p_sb, dg)

                av_q = []

                def emit_attn_PT(h, ib):
                    """scaled transpose of P for block ib; AV trails a step
                    so its pt_sb copies are long done when PE reaches it."""
                    p_sb, dg = stash.pop((h, ib))
                    pt_sb = pt_pool.tile([P, NCH, P], bf16, name="pt_sb", tag="pt_sb")
                    for g in range((NCH + PTG - 1) // PTG):
                        gn = min(PTG, NCH - g * PTG)
                        ptp = ps_pt.tile([P, gn * P], f32, name="ptp", tag="ptp")
                        for c in range(gn):
                            jc = g * PTG + c
                            nc.tensor.matmul(
                                ptp[:, c * P:(c + 1) * P],
                                p_sb[:, jc * P:(jc + 1) * P], dg,
                                start=True, stop=True)
                        dst = pt_sb[:, g * PTG:g * PTG + gn, :].rearrange(
                            "p a b -> p (a b)")
                        # ~25% of PT psum->sbuf moves on ACT, rest on DVE
                        if g == 0 and ib % 2 == 0:
                            nc.scalar.copy(dst, ptp)
                        else:
                            nc.vector.tensor_copy(dst, ptp)
                    av_q.append((h, ib, pt_sb))

                def emit_attn_AV():
                    h, ib, pt_sb = av_q.pop(0)
                    par = (h % HPT) * HS
                    av = ps_av.tile([HS, P], f32, name="av", tag="av")
                    for jc in range(NCH):
                        nc.tensor.matmul(
                            av,
                            v_sb[jc][:, h * HS:(h + 1) * HS],
                            pt_sb[:, jc, :],
                            start=(jc == 0), stop=(jc == NCH - 1))
                    nc.vector.tensor_copy(
                        attnT_sb[h // HPT][par:par + HS, ib * P:(ib + 1) * P],
                        av)
                    if h == HCL - 1:
                        emit_out_block(ib)

                # all input loads issued up-front, spread across HWDGE rings so
                # later a_dr/skew/y traffic queues behind nothing compute-bound
                wv_all, wv_sb, wv_frees = alloc_w("wv")
                wk_all, wk_sb, wk_frees = alloc_w("wk")
                # wq per-chunk (overlaps with xT chunk arrival), rest batched
                wq_sb, wq_frees = [], []
                for kb in range(KC):
                    t, fr = tc.tile([P, D], bf16, name=f"wq_{kb}")
                    wq_sb.append(t)
                    wq_frees.append(fr)
                nc.scalar.dma_start(out=wq_sb[0], in_=wq_d[0:P, :])
                nc.scalar.dma_start(out=et_bf, in_=et_d[:, :])
                for kb in range(1, KC):
                    nc.scalar.dma_start(out=wq_sb[kb],
                                        in_=wq_d[kb * P:(kb + 1) * P, :])
                nc.sync.dma_start(out=wk_all[:, :, :],
                                  in_=fold_src(wk_d, KC, D))
                nc.scalar.dma_start(out=wv_all[:, :, :],
                                    in_=fold_src(wv_d, KC, D))
                nc.sync.dma_start(out=wo_all[:, :, :],
                                  in_=fold_src(wo_d, DT, C))

                for (wname, w_sb, w_frees, dest) in (
                        ("wq", wq_sb, wq_frees, qT_sb),
                        ("wk", wk_sb, wk_frees, kT_sb)):
                    for mb in range(DT):
                        raw, raw_free = tc.tile([P, T], bf16, name=f"raw{wname}_{mb}")
                        for nh in range(NJH):
                            pp = ps_pt.tile([P, JH], f32, name="projps", tag="ptp")
                            for kb in range(KC):
                                nc.tensor.matmul(
                                    pp,
                                    w_sb[kb][:, mb * P:(mb + 1) * P],
                                    xT_sb[kb][:, nh * JH:(nh + 1) * JH],
                                    start=(kb == 0), stop=(kb == KC - 1),
                                )
                            if wname == "wq":
                                nc.scalar.copy(
                                    raw[:, nh * JH:(nh + 1) * JH], pp)
                            else:
                                nc.vector.tensor_copy(
                                    raw[:, nh * JH:(nh + 1) * JH], pp)
                        # rotary: bf16 throughout; alternate DVE/Pool per tile
                        o = dest[mb]
                        rt = rot_tmps[mb % 2]
                        eng = nc.vector if mb % 2 == 0 else nc.gpsimd
                        nc.vector.stream_shuffle(rt, raw, shuf_mask)
                        eng.tensor_mul(o, raw, cos_sb)
                        eng.tensor_mul(rt, rt, sin_sb)
                        eng.tensor_add(o, o, rt)
                        raw_free()
                    if wname == "wq":
                        for ib in range(NB):
                            emit_A_block(0, ib)
                    for fr in reversed(w_frees):
                        fr()

                # v[t, d] (bf16), with block-0 score/softmax interleaved
                v_sb = [persist.tile([P, D], bf16, name=f"v_{tb}", tag=f"v{tb}")
                        for tb in range(NB)]
                vw = min(JH, D)
                for tb in range(NB):
                    for nh in range(max(D // JH, 1)):
                        pp = ps_pt.tile([P, vw], f32, name="vps", tag="ptp")
                        for kb in range(KC):
                            nc.tensor.matmul(
                                pp,
                                xT_sb[kb][:, tb * P:(tb + 1) * P],
                                wv_sb[kb][:, nh * vw:(nh + 1) * vw],
                                start=(kb == 0), stop=(kb == KC - 1),
                            )
                        nc.vector.tensor_copy(
                            v_sb[tb][:, nh * vw:(nh + 1) * vw], pp)
                    if tb >= 1:
                        emit_attn_SE(0, tb - 1)
                    emit_A_block(1, tb)
                for fr in reversed(wv_frees):
                    fr()
                rot_tmp1_free()
                rot_tmp0_free()
                for fr in reversed(xT_free):
                    fr()

                # ---------------- attention ----------------
                attnT_sb = [persist.tile([P, T], bf16, name=f"attnT_{mb}", tag=f"aT{mb}")
                            for mb in range(DT)]

                def emit_out_block(tb):
                    """final projection for row-block tb (all heads done)."""
                    for ch in range(max(C // JH, 1)):
                        cw = min(JH, C)
                        op = ps_pt.tile([P, cw], f32, name="ops", tag="ptp")
                        for hc in range(DT):
                            nc.tensor.matmul(
                                op,
                                attnT_sb[hc][:, tb * P:(tb + 1) * P],
                                wo_sb[hc][:, ch * cw:(ch + 1) * cw],
                                start=(hc == 0), stop=(hc == DT - 1))
                        o_sb = out_pool.tile([P, cw], bf16, name="o_sb", tag="o_sb")
                        nc.vector.tensor_copy(o_sb, op)
                        nc.scalar.dma_start(
                            out=y_d[tb * P:(tb + 1) * P, ch * cw:(ch + 1) * cw],
                            in_=o_sb)



                DELAY = 1
                for h in range(HCL):
                    for ib in range(NB):
                        tgt = h * NB + ib + 2 * NB
                        if tgt < HCL * NB:
                            emit_A_block(tgt // NB, tgt % NB)
                        if h > 0 or ib >= NB - 1:
                            emit_attn_SE(h, ib)
                        if ib >= DELAY:
                            emit_attn_PT(h, ib - DELAY)
                            if len(av_q) > 1:
                                emit_attn_AV()
                    for ib in range(NB - DELAY, NB):
                        emit_attn_PT(h, ib)
                        if len(av_q) > 1:
                            emit_attn_AV()
                while av_q:
                    emit_attn_AV()
                for fr in reversed(wo_frees):
                    fr()


    nc.compile()
    return nc


# partition p (within a head's 64) holds head-dim SIGMA[p]; pairs
# (d, d+32) land 16 apart inside a 32-partition quadrant.
SIGMA = np.concatenate([
    np.arange(0, 16), np.arange(32, 48),
    np.arange(16, 32), np.arange(48, 64),
])


def _host_tables(T, hs):
    inv_freq = 1.0 / (10000.0 ** (np.arange(0, hs, 2, dtype=np.float64) / hs))
    t = np.arange(T, dtype=np.float64)
    fr = np.outer(inv_freq, t)                     # [hs/2, T]
    cosT = np.empty((128, T), np.float32)
    sinS = np.empty((128, T), np.float32)
    for blk in range(128 // hs):
        for p in range(hs):
            d = SIGMA[p]
            row = blk * hs + p
            cosT[row] = np.cos(fr[d % 32]).astype(np.float32)
            s = np.sin(fr[d % 32]).astype(np.float32)
            sinS[row] = -s if d < 32 else s
    return cosT, sinS


def make_et8(E, T, scale=8.0):
    et8 = np.zeros((128, 2 * T), np.float32)
    etp = (scale * E.T[SIGMA]).astype(np.float32)   # [64, 2T-1] permuted rows
    et8[:HS, :E.shape[0]] = etp
    et8[HS:2 * HS, :E.shape[0]] = etp
    return et8


def perm_cols(w, D):
    """Permute per-head 64-column blocks of w [C, D] by SIGMA."""
    idx = (np.arange(D) // HS) * HS + SIGMA[np.arange(D) % HS]
    return np.ascontiguousarray(w[:, idx])


def get_nc(T=SEQ, C=EMB, D=HC * HS):
    key = (T, C, D)
    if key not in _cache:
        _cache[key] = _build_nc(T, C, D)
    return _cache[key]


def kernel(x, wq, wk, wv, wo, bo, rel_pos_emb):
    from concourse.bass_utils import run_bass_kernel_spmd

    x = np.asarray(x, dtype=np.float32)
    wq = np.asarray(wq, dtype=np.float32)
    wk = np.asarray(wk, dtype=np.float32)
    wv = np.asarray(wv, dtype=np.float32)
    wo = np.asarray(wo, dtype=np.float32)
    bo = np.asarray(bo, dtype=np.float32)
    E = np.asarray(rel_pos_emb, dtype=np.float32)

    T, C, D = SEQ, EMB, HC * HS
    nc = get_nc(T, C, D)

    cosT, sinS = _host_tables(T, HS)
    et8 = make_et8(E, T)

    import ml_dtypes
    bf16 = ml_dtypes.bfloat16
    in_maps = []
    for core in range(N_CORES):
        b, g = divmod(core, 2)
        sl = slice(g * D, (g + 1) * D)
        in_maps.append({
            "xT": np.ascontiguousarray(x[b].T).astype(bf16),
            "wq": (perm_cols(wq[:, sl], D) * np.float32(0.125)).astype(bf16),
            "wk": perm_cols(wk[:, sl], D).astype(bf16),
            "wv": np.ascontiguousarray(wv[:, sl]).astype(bf16),
            "wo": np.ascontiguousarray(wo[sl, :]).astype(bf16),
            "csT": np.concatenate([cosT, sinS], axis=1).astype(bf16),
            "et8": et8.astype(bf16),
        })

    res = run_bass_kernel_spmd(nc, in_maps, core_ids=list(range(N_CORES)))
    out = np.empty((BATCH, T, C), np.float32)
    for b in range(BATCH):
        out[b] = (np.asarray(res.results[2 * b]["y"], np.float32)
                  + np.asarray(res.results[2 * b + 1]["y"], np.float32) + bo)
    return out

